# revision 1
# baseline (speedup 1.0000x reference)
"""ConvMultiheadAttention Trainium2 kernel (8 NeuronCores).

Sharding: core c = (batch b = c//2) x (head-group hg = c%2, 8 heads each).
Per core:
  - q/k/v conv1d projections (K=3, same pad) for this core's 512 output
    channels, expressed as PSUM-accumulated bf16 matmuls over x laid out
    [c_in partitions, L free].
  - attention with TRANSPOSED scores sT[j, i] (j on partitions) so the
    key-padding mask folds into the Exp activation's per-partition bias,
    and the softmax denominator comes for free from a ones-augmented
    AV matmul (row 64 of the [65, i] psum = column sums).
  - partial out-conv contracting over this core's 512 attention-output
    channels; the host sums the two partials per batch.
Host folds: attention scale + q-bias into q-conv weights/bias; k-bias is
dropped (constant per softmax row -> cancels); v-bias and o-bias are
applied on the host after the gather (attention rows sum to 1).
"""

import os
import numpy as np
import ml_dtypes

BF16 = ml_dtypes.bfloat16

B, L, D = 4, 1024, 1024
NH, HD = 16, 64
KW = 3
NCORES = 8
HALF = D // 2  # channels per core half (8 heads)
SCALE = HD ** -0.5
MASK_BIAS = -30000.0

_CACHE = {}


def _build_nc():
    import concourse.bass as bass  # noqa: F401
    import concourse.tile as tile
    from concourse import bacc, mybir

    f32 = mybir.dt.float32
    bf16 = mybir.dt.bfloat16
    Act = mybir.ActivationFunctionType

    nc = bacc.Bacc(
        "TRN2",
        target_bir_lowering=False,
        debug=False,
        enable_asserts=False,
        num_devices=NCORES,
    )

    # ---- DRAM I/O ----
    xq_d = nc.dram_tensor("xq", [8, 128, L], bf16, kind="ExternalInput").ap()
    xk_d = nc.dram_tensor("xk", [8, 128, L], bf16, kind="ExternalInput").ap()
    xv_d = nc.dram_tensor("xv", [8, 128, L], bf16, kind="ExternalInput").ap()
    wq_d = nc.dram_tensor("wq", [4, 128, KW, 8, 128], bf16, kind="ExternalInput").ap()
    wk_d = nc.dram_tensor("wk", [4, 128, KW, 8, 128], bf16, kind="ExternalInput").ap()
    wv_d = nc.dram_tensor("wv", [4, 128, KW, 8, 128], bf16, kind="ExternalInput").ap()
    wo_d = nc.dram_tensor("wo", [8, 128, KW, 4, 128], bf16, kind="ExternalInput").ap()
    qb_d = nc.dram_tensor("qb", [128, 4], f32, kind="ExternalInput").ap()
    jb_d = nc.dram_tensor("jb", [128, 8], f32, kind="ExternalInput").ap()
    out_d = nc.dram_tensor("out", [8, 128, L], f32, kind="ExternalOutput").ap()

    from concourse.masks import make_identity

    with tile.TileContext(nc) as tc:
        with (
            tc.tile_pool(name="singles", bufs=1) as singles,
            tc.tile_pool(name="wpool", bufs=3) as wpool,
            tc.tile_pool(name="qk", bufs=2) as qkpool,
            tc.tile_pool(name="vpool", bufs=2) as vpool,
            tc.tile_pool(name="ppool", bufs=2) as ppool,
            tc.tile_pool(name="outp", bufs=4) as outp,
            tc.tile_pool(name="smalls", bufs=4) as smalls,
            tc.tile_pool(name="convp", bufs=2, space="PSUM") as convp,
            tc.tile_pool(name="scorep", bufs=2, space="PSUM") as scorep,
            tc.tile_pool(name="avp", bufs=2, space="PSUM") as avp,
        ):
            # ---- constants / resident tiles ----
            ident = singles.tile([128, 128], bf16, tag="ident")
            make_identity(nc, ident)
            qb_s = singles.tile([128, 4], f32, tag="qb")
            nc.sync.dma_start(qb_s, qb_d)
            jb_s = singles.tile([128, 8], f32, tag="jb")
            nc.sync.dma_start(jb_s, jb_d)

            xq_s = singles.tile([128, 8, L], bf16, tag="xq")
            xk_s = singles.tile([128, 8, L], bf16, tag="xk")
            xv_s = singles.tile([128, 8, L], bf16, tag="xv")
            for cc in range(8):
                nc.sync.dma_start(xv_s[:, cc, :], xv_d[cc])
            for cc in range(8):
                nc.sync.dma_start(xq_s[:, cc, :], xq_d[cc])
            for cc in range(8):
                nc.sync.dma_start(xk_s[:, cc, :], xk_d[cc])

            # vT[j_part, j_chunk, head, 0:64] = v[h*64+d, j]; col 64 = ones
            vT = singles.tile([128, 8, 8, 65], bf16, tag="vT")
            for h in range(8):
                nc.vector.memset(vT[:, :, h, 64:65], 1.0)
            o_x = singles.tile([128, 4, L], bf16, tag="ox")

            def conv_mms(ps, w_t, x_t, n_ci, lh):
                """Accumulate conv-as-matmul into psum ps[:, 0:512] for
                output columns [lh*512, lh*512+512)."""
                # center tap first: full-width start=True write covers the
                # whole bank, so the edge taps' partial-width writes are
                # pure accumulations (uniform has_written state).
                korder = [1, 0, 2]
                first = True
                for k in korder:
                    for cc in range(n_ci):
                        lo = lh * 512 + k - 1
                        lhsT = w_t[:, k, cc, :]
                        if lo < 0:
                            rhs = x_t[:, cc, 0:511]
                            outap = ps[:, 1:512]
                        elif lo + 512 > L:
                            rhs = x_t[:, cc, lo:L]
                            outap = ps[:, 0 : L - lo]
                        else:
                            rhs = x_t[:, cc, lo : lo + 512]
                            outap = ps[:, 0:512]
                        nc.tensor.matmul(
                            outap,
                            lhsT,
                            rhs,
                            start=first,
                            stop=(k == korder[-1] and cc == n_ci - 1),
                        )
                        first = False

            # ---- V conv + transpose into vT ----
            for occ in range(4):
                wv_t = wpool.tile([128, KW, 8, 128], bf16, tag="w")
                nc.sync.dma_start(wv_t, wv_d[occ])
                v_t = vpool.tile([128, L], bf16, tag="v")
                for lh in range(2):
                    ps = convp.tile([128, 512], f32, tag="cp")
                    conv_mms(ps, wv_t, xv_s, 8, lh)
                    nc.vector.tensor_copy(v_t[:, lh * 512 : (lh + 1) * 512], ps)
                for lb in range(8):
                    tp = convp.tile([128, 128], bf16, tag="cp")
                    nc.tensor.transpose(tp, v_t[:, lb * 128 : (lb + 1) * 128], ident)
                    nc.vector.tensor_copy(vT[:, lb, 2 * occ, 0:64], tp[:, 0:64])
                    nc.vector.tensor_copy(vT[:, lb, 2 * occ + 1, 0:64], tp[:, 64:128])

            # ---- per head-pair: software-pipelined q/k conv + attention ----
            # Per pair t: scores+exp(t) -> q/k conv(t+1) [PE work that hides
            # exp(t) on ACT] -> AV+normalize(t).
            def qk_conv_units(t):
                """Return (q_t, k_t, units): four closures each emitting one
                conv psum-group (~5us of dense PE work) for pair t."""
                q_t = qkpool.tile([128, L], bf16, tag="q", name=f"q{t}")
                k_t = qkpool.tile([128, L], bf16, tag="k", name=f"k{t}")
                state = {}

                def unit(which, lh):
                    if which == "q" and lh == 0:
                        state["wq"] = wpool.tile([128, KW, 8, 128], bf16, tag="w",
                                                 name="wqt")
                        nc.sync.dma_start(state["wq"], wq_d[t])
                    if which == "k" and lh == 0:
                        state["wk"] = wpool.tile([128, KW, 8, 128], bf16, tag="w",
                                                 name="wkt")
                        nc.sync.dma_start(state["wk"], wk_d[t])
                    ps = convp.tile([128, 512], f32, tag="cp")
                    if which == "q":
                        conv_mms(ps, state["wq"], xq_s, 8, lh)
                        nc.vector.tensor_scalar_add(
                            q_t[:, lh * 512 : (lh + 1) * 512], ps,
                            qb_s[:, t : t + 1])
                    else:
                        conv_mms(ps, state["wk"], xk_s, 8, lh)
                        nc.vector.tensor_copy(
                            k_t[:, lh * 512 : (lh + 1) * 512], ps)

                units = [lambda w=w, lh=lh: unit(w, lh)
                         for w in ("q", "k") for lh in range(2)]
                return q_t, k_t, units

            def qk_conv(t):
                q_t, k_t, units = qk_conv_units(t)
                for u in units:
                    u()
                return q_t, k_t

            _ablate = os.environ.get("KERNEL_ABLATE", "")
            q_t, k_t = qk_conv(0)
            if _ablate == "noattn":
                nc.vector.memset(o_x, 0.01)
                for t in range(1, 4):
                    q_t, k_t = qk_conv(t)
            for t in range(4 if _ablate != "noattn" else 0):
                # next pair's conv psum-groups, interleaved between score
                # jc-groups below so PE stays busy while ACT runs exp(t)
                if t < 3:
                    nq_t, nk_t, conv_units = qk_conv_units(t + 1)
                else:
                    conv_units = []
                # scores + exp for both heads; adjacent matmuls of the two
                # heads hit disjoint PE row groups (base 0 / 64) and overlap.
                p_pair = []
                for jc in range(8):
                    sps_pair = [scorep.tile([128, L], f32, tag="score",
                                            name=f"sps{hh2}")
                                for hh2 in range(2)]
                    if jc == 0:
                        p_pair = [ppool.tile([128, 8, L], bf16, tag="p",
                                             name=f"p{hh2}")
                                  for hh2 in range(2)]
                    for ih in range(2):
                        for hh in range(2):
                            base = hh * 64
                            nc.tensor.matmul(
                                sps_pair[hh][:, ih * 512 : (ih + 1) * 512],
                                k_t[base : base + 64, jc * 128 : (jc + 1) * 128],
                                q_t[base : base + 64, ih * 512 : (ih + 1) * 512],
                                start=True,
                                stop=True,
                            )
                    for hh in range(2):
                        nc.scalar.activation(
                            p_pair[hh][:, jc, :], sps_pair[hh], Act.Exp,
                            bias=jb_s[:, jc : jc + 1],
                        )
                    # one conv psum-group (~5us dense PE) after every other
                    # jc-group: fills the PE stall while ACT drains exp(t)
                    if jc % 2 == 1 and conv_units:
                        conv_units.pop(0)()
                # AV + normalize for both heads
                for hh in range(2):
                    h = 2 * t + hh
                    base = hh * 64
                    for ih in range(2):
                        avps = avp.tile([65, 512], f32, tag="av")
                        for jc in range(8):
                            nc.tensor.matmul(
                                avps,
                                vT[:, jc, h, :],
                                p_pair[hh][:, jc, ih * 512 : (ih + 1) * 512],
                                start=(jc == 0),
                                stop=(jc == 7),
                            )
                        r_t = smalls.tile([1, 512], f32, tag="r")
                        nc.vector.reciprocal(r_t, avps[64:65, :])
                        bc_t = smalls.tile([64, 512], f32, tag="bc")
                        nc.gpsimd.partition_broadcast(bc_t, r_t)
                        dst = o_x[base : base + 64, t, ih * 512 : (ih + 1) * 512]
                        if hh == 0:
                            nc.vector.tensor_mul(dst, avps[0:64, :], bc_t)
                        else:
                            tmp = smalls.tile([64, 512], bf16, tag="tmp")
                            nc.vector.tensor_mul(tmp, avps[0:64, :], bc_t)
                            nc.sync.dma_start(dst, tmp)
                if t < 3:
                    q_t, k_t = nq_t, nk_t

            # ---- out conv (partial over this core's 512 input channels) ----
            for occ in range(8 if _ablate != "nooconv" else 0):
                wo_t = wpool.tile([128, KW, 4, 128], bf16, tag="w")
                nc.sync.dma_start(wo_t, wo_d[occ])
                for lh in range(2):
                    ps = convp.tile([128, 512], f32, tag="cp")
                    conv_mms(ps, wo_t, o_x, 4, lh)
                    o_t = outp.tile([128, 512], f32, tag="osb")
                    nc.vector.tensor_copy(o_t, ps)
                    nc.sync.dma_start(out_d[occ, :, lh * 512 : (lh + 1) * 512], o_t)

    nc.compile()
    return nc


def _get_nc():
    if "nc" not in _CACHE:
        _CACHE["nc"] = _build_nc()
    return _CACHE["nc"]


def _prep_inputs(query, key, value, key_padding_mask, attn_mask,
                 q_w, q_b, k_w, k_b, v_w, v_b, o_w, o_b):
    """Build the 8 per-core input maps (host-side shard + layout)."""
    query = np.asarray(query, np.float32)
    key = np.asarray(key, np.float32)
    value = np.asarray(value, np.float32)
    kpm = np.asarray(key_padding_mask)
    attn_mask = np.asarray(attn_mask, np.float32)
    q_w = np.asarray(q_w, np.float32); q_b = np.asarray(q_b, np.float32)
    k_w = np.asarray(k_w, np.float32)
    v_w = np.asarray(v_w, np.float32)
    o_w = np.asarray(o_w, np.float32); o_b = np.asarray(o_b, np.float32)

    # attn_mask must be constant across query rows to fold into the key bias
    if not np.all(attn_mask == attn_mask[0:1, :]):
        raise NotImplementedError("attn_mask varying over query index unsupported")
    am_row = attn_mask[0]

    def conv_w_layout(w, occ, n_ci):
        # w: [C_out_part, C_in_part, KW] -> [occ, p(ci), k, cc, m(c_out)]
        co, ci, _ = w.shape
        arr = w.reshape(occ, 128, n_ci, 128, KW).transpose(0, 3, 4, 2, 1)
        return np.ascontiguousarray(arr).astype(BF16)

    wq_h, wk_h, wv_h, wo_h, qb_h = [], [], [], [], []
    for hg in range(2):
        sl = slice(hg * HALF, (hg + 1) * HALF)
        wq_h.append(conv_w_layout(q_w[sl] * SCALE, 4, 8))
        wk_h.append(conv_w_layout(k_w[sl], 4, 8))
        wv_h.append(conv_w_layout(v_w[sl], 4, 8))
        # out conv: contract over this half's input channels
        wo_h.append(conv_w_layout(o_w[:, sl, :], 8, 4))
        qb_h.append(np.ascontiguousarray(
            (q_b[sl] * SCALE).reshape(4, 128).T).astype(np.float32))

    xq_b, xk_b, xv_b, jb_b = [], [], [], []
    for b in range(B):
        xq_b.append(np.ascontiguousarray(query[b].T).reshape(8, 128, L).astype(BF16))
        xk_b.append(np.ascontiguousarray(key[b].T).reshape(8, 128, L).astype(BF16))
        xv_b.append(np.ascontiguousarray(value[b].T).reshape(8, 128, L).astype(BF16))
        jb = np.where(kpm[b], MASK_BIAS, 0.0).astype(np.float32) + am_row
        jb_b.append(np.ascontiguousarray(jb.reshape(8, 128).T).astype(np.float32))

    in_maps = []
    for c in range(NCORES):
        b, hg = c // 2, c % 2
        in_maps.append({
            "xq": xq_b[b], "xk": xk_b[b], "xv": xv_b[b],
            "wq": wq_h[hg], "wk": wk_h[hg], "wv": wv_h[hg], "wo": wo_h[hg],
            "qb": qb_h[hg], "jb": jb_b[b],
        })
    return in_maps, (o_w, np.asarray(v_b, np.float32), o_b)


def _postprocess(parts, extras):
    """parts: list of 8 arrays [8,128,L] f32 -> full output [B, L, D] f32."""
    o_w, v_b, o_b = extras
    # v-bias contribution through the out conv (attention rows sum to 1):
    # interior columns see all 3 taps, edge columns lose one.
    a_full = o_w.sum(axis=2) @ v_b            # [D]
    a_l0 = a_full - o_w[:, :, 0] @ v_b        # l = 0 loses tap k=0
    a_lL = a_full - o_w[:, :, 2] @ v_b        # l = L-1 loses tap k=2
    out = np.empty((B, L, D), np.float32)
    for b in range(B):
        tot = (parts[2 * b] + parts[2 * b + 1]).reshape(D, L)
        tot = tot + o_b[:, None] + a_full[:, None]
        tot[:, 0] += a_l0 - a_full
        tot[:, -1] += a_lL - a_full
        out[b] = tot.T
    return out


def _run(in_maps, trace=False, **kw):
    from concourse import bass_utils
    nc = _get_nc()
    try:
        res = bass_utils.run_bass_kernel_spmd(
            nc, in_maps, core_ids=list(range(NCORES)), trace=trace, **kw)
    except ModuleNotFoundError:
        # NTFF profiling hook unavailable (axon client without axon.trn);
        # rerun without trace.
        res = bass_utils.run_bass_kernel_spmd(
            nc, in_maps, core_ids=list(range(NCORES)), trace=False, **kw)
    return res


def kernel(**inputs) -> np.ndarray:
    in_maps, extras = _prep_inputs(**inputs)
    res = _run(in_maps, trace=bool(int(os.environ.get("KERNEL_TRACE", "0"))))
    parts = [res.results[c]["out"] for c in range(NCORES)]
    out = _postprocess(parts, extras)
    if res.exec_time_ns is not None:
        print(f"HW exec time: {res.exec_time_ns} ns")
    return out



# revision 13
# speedup vs baseline: 1.1464x; 1.1464x over previous
"""ConvMultiheadAttention Trainium2 kernel (8 NeuronCores).

Sharding: core c = (batch b = c//2) x (head-group hg = c%2, 8 heads each).

All four convs run as fp8(e4m3) DoubleRow matmuls (256-wide contraction at
0.5 PE-cycles/row) with hi+lo residual splits of both weights and
activations for ~bf16 accuracy at 0.75x the bf16 PE cost:
  w*x ~= w_hi*x_hi + w_hi*x_lo + w_lo*x_hi   (lo terms at the same
  power-of-2 scale as hi, so all products accumulate in one psum group).
The V conv runs in transposed form (out[j, co]) so vT needs no PE
transposes. Scores are fp8 DoubleRow with q fully compensated (hi/lo as
the two k-tiles) and k raw-fp8 (hi duplicated): 2x cheaper than bf16,
~1% output error. AV + softmax stay bf16 (ones-row denominator trick).
The out conv is split into two cc-halves: the pairs-0/1 half runs inside
pair 3's exp window (PE filler), accumulating into SBUF; the pairs-2/3
half adds it back during the final psum->sbuf copy.

Host folds: attention scale + q-bias into q-conv weights/bias (bias enters
the conv psum via a ones-row bf16 matmul); k-bias dropped (softmax-
invariant); v-bias and o-bias applied on host after the gather.
"""

import os
import numpy as np
import ml_dtypes

BF16 = ml_dtypes.bfloat16
F8 = ml_dtypes.float8_e4m3

B, L, D = 4, 1024, 1024
NH, HD = 16, 64
KW = 3
NCORES = 8
HALF = D // 2  # channels per core half (8 heads)
SCALE = HD ** -0.5
MASK_BIAS = -30000.0

# power-of-2 scales: x inputs at 2^1; conv weights chosen so psum values
# land near sigma~37 (e4m3 max 240, so ~6 sigma of range)
SX = 2.0
SWQ = 2.0 ** 8   # on q_w * SCALE
SWK = 2.0 ** 5
SWV = 2.0 ** 5
SWO = 2.0 ** 5
SQ = SWQ * SX        # 2^9: scale of q values in fp8
SK = SWK * SX        # 2^6: scale of k values in fp8
SV = SWV * SX        # 2^6: scale of v / attention-output values
EXP_SCALE = 1.0 / (SQ * SK)   # 2^-15
OUT_SCALE = 1.0 / (SV * SWO)  # 2^-11: final psum -> true output

_CACHE = {}


def _build_nc():
    import concourse.bass as bass  # noqa: F401
    import concourse.tile as tile
    from concourse import bacc, mybir

    f32 = mybir.dt.float32
    bf16 = mybir.dt.bfloat16
    fp8 = mybir.dt.float8e4
    Act = mybir.ActivationFunctionType
    DR = mybir.MatmulPerfMode.DoubleRow
    Alu = mybir.AluOpType

    nc = bacc.Bacc(
        "TRN2",
        target_bir_lowering=False,
        debug=False,
        enable_asserts=False,
        num_devices=NCORES,
    )

    # ---- DRAM I/O ----
    def din(name, shape, dt=fp8):
        return nc.dram_tensor(name, shape, dt, kind="ExternalInput").ap()

    xq_d = [din("xqh", [8, 128, L]), din("xql", [8, 128, L])]
    xk_d = [din("xkh", [8, 128, L]), din("xkl", [8, 128, L])]
    xv_d = [din("xvh", [8, 128, L]), din("xvl", [8, 128, L])]
    wq_d = [din("wqh", [4, 128, KW, 8, 128]), din("wql", [4, 128, KW, 8, 128])]
    wk_d = [din("wkh", [4, 128, KW, 8, 128]), din("wkl", [4, 128, KW, 8, 128])]
    wv_d = [din("wvh", [4, 128, KW, 8, 128]), din("wvl", [4, 128, KW, 8, 128])]
    wo_d = [din("woh", [8, 128, KW, 4, 128]), din("wol", [8, 128, KW, 4, 128])]
    qb_d = din("qb", [1, 512], bf16)
    jb_d = din("jb", [128, 8], f32)
    out_d = nc.dram_tensor("out", [8, 128, L], f32, kind="ExternalOutput").ap()

    LP = L + 2  # padded length: col 0 and L+1 are zeros, data at 1..L

    with tile.TileContext(nc) as tc:
        with (
            tc.tile_pool(name="singles", bufs=1) as singles,
            tc.tile_pool(name="wpool", bufs=2) as wpool,
            tc.tile_pool(name="qk", bufs=2) as qkpool,
            tc.tile_pool(name="ppool", bufs=2) as ppool,
            tc.tile_pool(name="outp", bufs=4) as outp,
            tc.tile_pool(name="smalls", bufs=2) as smalls,
            tc.tile_pool(name="convp", bufs=2, space="PSUM") as convp,
            tc.tile_pool(name="scorep", bufs=2, space="PSUM") as scorep,
            tc.tile_pool(name="avp", bufs=2, space="PSUM") as avp,
        ):
            # ---- resident tiles ----
            qb_s = singles.tile([1, 512], bf16, tag="qb")
            nc.sync.dma_start(qb_s, qb_d)
            jb_s = singles.tile([128, 8], f32, tag="jb")
            nc.sync.dma_start(jb_s, jb_d)
            ones = singles.tile([1, 512], bf16, tag="ones")
            nc.vector.memset(ones, 1.0)

            def xload(tag, dpair):
                hi = singles.tile([128, 8, LP], fp8, tag=tag + "h", name=tag + "h")
                lo = singles.tile([128, 8, LP], fp8, tag=tag + "l", name=tag + "l")
                for t in (hi, lo):
                    nc.vector.memset(t[:, :, 0:1], 0.0)
                    nc.vector.memset(t[:, :, L + 1 : L + 2], 0.0)
                for t, d in ((hi, dpair[0]), (lo, dpair[1])):
                    for cc in range(8):
                        nc.sync.dma_start(t[:, cc, 1 : L + 1], d[cc])
                return hi, lo

            xv_s = xload("xv", xv_d)
            xq_s = xload("xq", xq_d)
            xk_s = xload("xk", xk_d)

            # vT[j_part, jc, head, 0:64] = v[h*64+d, j] * SV; col 64 = ones
            vT = singles.tile([128, 8, 8, 65], bf16, tag="vT")
            for h in range(8):
                nc.vector.memset(vT[:, :, h, 64:65], 1.0)
            from concourse.masks import make_identity
            ident = singles.tile([128, 128], bf16, tag="ident")
            make_identity(nc, ident)

            ox_h = singles.tile([128, 4, LP], fp8, tag="oxh")
            ox_l = singles.tile([128, 4, LP], fp8, tag="oxl")
            for t in (ox_h, ox_l):
                nc.vector.memset(t[:, :, 0:1], 0.0)
                nc.vector.memset(t[:, :, L + 1 : L + 2], 0.0)

            sumA = singles.tile([128, 16, 512], bf16, tag="sumA")

            def conv_dr(ps, wh, wl, xh, xl, lh, bias_lhsT=None):
                """512-col direct conv psum group: out cols [lh*512,+512),
                contraction 8 cc x 3 taps via fp8 DoubleRow hi/lo terms."""
                mms = []
                for k in (1, 0, 2):
                    base = lh * 512 + k
                    for c in range(0, 8, 2):
                        for wt, xt in ((wh, xh), (wh, xl), (wl, xh)):
                            mms.append((wt[:, k, c : c + 2, :],
                                        xt[:, c : c + 2, base : base + 512]))
                for i, (w_ap, x_ap) in enumerate(mms):
                    last = i == len(mms) - 1 and bias_lhsT is None
                    nc.tensor.matmul(ps, w_ap, x_ap, start=(i == 0),
                                     stop=last, perf_mode=DR)
                if bias_lhsT is not None:
                    nc.tensor.matmul(ps, bias_lhsT, ones, start=False,
                                     stop=True)

            # ---- V conv (direct, DR fp8) + PE transpose into vT ----
            for occ in range(4):
                wv_h = wpool.tile([128, KW, 8, 128], fp8, tag="wvh",
                                  name="wvht")
                wv_l = wpool.tile([128, KW, 8, 128], fp8, tag="wvl",
                                  name="wvlt")
                nc.sync.dma_start(wv_h, wv_d[0][occ])
                nc.sync.dma_start(wv_l, wv_d[1][occ])
                v_t = qkpool.tile([128, L], bf16, tag="v", name="vt")
                for lh in range(2):
                    ps = convp.tile([128, 512], f32, tag="cp")
                    conv_dr(ps, wv_h, wv_l, xv_s[0], xv_s[1], lh)
                    nc.vector.tensor_copy(v_t[:, lh * 512 : (lh + 1) * 512],
                                          ps)
                for lb in range(8):
                    tp = convp.tile([128, 128], bf16, tag="cp")
                    nc.tensor.transpose(tp, v_t[:, lb * 128 : (lb + 1) * 128],
                                        ident)
                    nc.vector.tensor_copy(vT[:, lb, 2 * occ, 0:64],
                                          tp[:, 0:64])
                    nc.vector.tensor_copy(vT[:, lb, 2 * occ + 1, 0:64],
                                          tp[:, 64:128])

            # ---- per head-pair: pipelined q/k conv + attention ----
            def qk_conv_units(t):
                """(q_t, k_t, units): 4 closures each emitting one conv
                psum-group (~4us dense PE work) for pair t.
                q_t[:, 0, :] = q_hi, [:, 1, :] = q_lo (DoubleRow k-tiles);
                k_t[:, 0, :] = k_t[:, 1, :] = k_hi."""
                q_t = qkpool.tile([128, 2, L], fp8, tag="q", name=f"q{t}")
                # k: [ch, jc, ktile, m] so the scores stationary slice
                # k_t[d, jc, :, :] is ISA-contiguous (2, 128)
                k_t = qkpool.tile([128, 8, 2, 128], fp8, tag="k", name=f"k{t}")
                state = {}

                def unit(which, lh):
                    if which == "q" and lh == 0:
                        state["wqh"] = wpool.tile([128, KW, 8, 128], fp8,
                                                  tag="wqh", name="wqht")
                        state["wql"] = wpool.tile([128, KW, 8, 128], fp8,
                                                  tag="wql", name="wqlt")
                        nc.sync.dma_start(state["wqh"], wq_d[0][t])
                        nc.sync.dma_start(state["wql"], wq_d[1][t])
                    if which == "k" and lh == 0:
                        state["wkh"] = wpool.tile([128, KW, 8, 128], fp8,
                                                  tag="wkh", name="wkht")
                        state["wkl"] = wpool.tile([128, KW, 8, 128], fp8,
                                                  tag="wkl", name="wklt")
                        nc.sync.dma_start(state["wkh"], wk_d[0][t])
                        nc.sync.dma_start(state["wkl"], wk_d[1][t])
                    if which == "q":
                        ps = convp.tile([128, 512], f32, tag="cp")
                        sl = slice(lh * 512, (lh + 1) * 512)
                        conv_dr(ps, state["wqh"], state["wql"], xq_s[0],
                                xq_s[1], lh,
                                bias_lhsT=qb_s[0:1, t * 128 : (t + 1) * 128])
                        nc.vector.tensor_copy(q_t[:, 0, sl], ps)
                        nc.vector.tensor_sub(q_t[:, 1, sl], ps, q_t[:, 0, sl])
                    else:
                        ps = convp.tile([128, 4, 128], f32, tag="cp")
                        conv_dr(ps, state["wkh"], state["wkl"], xk_s[0],
                                xk_s[1], lh)
                        jsl = slice(lh * 4, (lh + 1) * 4)
                        nc.vector.tensor_copy(k_t[:, jsl, 0, :], ps)
                        nc.vector.tensor_copy(k_t[:, jsl, 1, :], ps)

                units = [lambda w=w, lh=lh: unit(w, lh)
                         for w in ("q", "k") for lh in range(2)]
                return q_t, k_t, units

            def qk_conv(t):
                q_t, k_t, units = qk_conv_units(t)
                for u in units:
                    u()
                return q_t, k_t

            # out-conv halves: cc pair (0,1) = head-pairs 0-1, (2,3) = 2-3
            wo_tiles = {}

            def out_half_unit(occ, lh, cp0, first_half):
                """One psum group: out[occ, lh*512:+512] partial over o_x
                cc chunks [cp0, cp0+2)."""
                key = (occ, "h" if first_half else "H")
                if key not in wo_tiles:
                    woh = wpool.tile([128, KW, 4, 128], fp8, tag="woh",
                                     name="woht")
                    wol = wpool.tile([128, KW, 4, 128], fp8, tag="wol",
                                     name="wolt")
                    nc.sync.dma_start(woh, wo_d[0][occ])
                    nc.sync.dma_start(wol, wo_d[1][occ])
                    wo_tiles[key] = (woh, wol)
                woh, wol = wo_tiles[key]
                ps = convp.tile([128, 512], f32, tag="cp")
                mms = []
                for k in (1, 0, 2):
                    base = lh * 512 + k
                    for wt, xt in ((woh, ox_h), (woh, ox_l), (wol, ox_h)):
                        mms.append((wt[:, k, cp0 : cp0 + 2, :],
                                    xt[:, cp0 : cp0 + 2, base : base + 512]))
                for i, (w_ap, x_ap) in enumerate(mms):
                    nc.tensor.matmul(ps, w_ap, x_ap, start=(i == 0),
                                     stop=(i == len(mms) - 1), perf_mode=DR)
                idx = occ * 2 + lh
                if first_half:
                    nc.vector.tensor_scalar_mul(sumA[:, idx, :], ps,
                                                OUT_SCALE)
                else:
                    o_t = outp.tile([128, 512], f32, tag="osb")
                    nc.vector.scalar_tensor_tensor(
                        o_t, ps, OUT_SCALE, sumA[:, idx, :],
                        Alu.mult, Alu.add)
                    nc.sync.dma_start(
                        out_d[occ, :, lh * 512 : (lh + 1) * 512], o_t)

            q_t, k_t = qk_conv(0)
            for t in range(4):
                if t < 3:
                    nq_t, nk_t, conv_units = qk_conv_units(t + 1)
                else:
                    conv_units = [
                        (lambda occ=occ, lh=lh:
                         out_half_unit(occ, lh, 0, True))
                        for occ in range(8) for lh in range(2)]
                p_pair = []
                for jc in range(8):
                    sps_pair = [scorep.tile([128, L], f32, tag="score",
                                            name=f"sps{hh2}")
                                for hh2 in range(2)]
                    if jc == 0:
                        p_pair = [ppool.tile([128, 8, L], bf16, tag="p",
                                             name=f"p{hh2}")
                                  for hh2 in range(2)]
                    for ih in range(2):
                        for hh in range(2):
                            base = hh * 64
                            nc.tensor.matmul(
                                sps_pair[hh][:, ih * 512 : (ih + 1) * 512],
                                k_t[base : base + 64, jc, :, :],
                                q_t[base : base + 64, :,
                                    ih * 512 : (ih + 1) * 512],
                                start=True, stop=True, perf_mode=DR,
                            )
                    for hh in range(2):
                        nc.scalar.activation(
                            p_pair[hh][:, jc, :], sps_pair[hh], Act.Exp,
                            bias=jb_s[:, jc : jc + 1], scale=EXP_SCALE,
                        )
                    if t < 3:
                        # one conv psum-group after every other jc-group
                        if jc % 2 == 1 and conv_units:
                            conv_units.pop(0)()
                    else:
                        # two out-conv first-half groups per jc
                        for _ in range(2):
                            if conv_units:
                                conv_units.pop(0)()
                # AV + normalize + fp8 hi/lo split of o_x
                for hh in range(2):
                    h = 2 * t + hh
                    base = hh * 64
                    for ih in range(2):
                        avps = avp.tile([65, 512], f32, tag="av")
                        for jc in range(8):
                            nc.tensor.matmul(
                                avps,
                                vT[:, jc, h, :],
                                p_pair[hh][:, jc,
                                           ih * 512 : (ih + 1) * 512],
                                start=(jc == 0), stop=(jc == 7),
                            )
                        r_t = smalls.tile([1, 512], f32, tag="r")
                        nc.vector.reciprocal(r_t, avps[64:65, :])
                        bc_t = smalls.tile([64, 512], f32, tag="bc")
                        nc.gpsimd.partition_broadcast(bc_t, r_t)
                        tmp = smalls.tile([64, 512], bf16, tag="tmp")
                        nc.vector.tensor_mul(tmp, avps[0:64, :], bc_t)
                        csl = slice(1 + ih * 512, 1 + (ih + 1) * 512)
                        if hh == 0:
                            nc.vector.tensor_copy(ox_h[0:64, t, csl], tmp)
                            nc.vector.tensor_sub(ox_l[0:64, t, csl], tmp,
                                                 ox_h[0:64, t, csl])
                        else:
                            thi = smalls.tile([64, 512], fp8, tag="thi")
                            tlo = smalls.tile([64, 512], fp8, tag="tlo")
                            nc.vector.tensor_copy(thi, tmp)
                            nc.vector.tensor_sub(tlo, tmp, thi)
                            nc.sync.dma_start(ox_h[64:128, t, csl], thi)
                            nc.sync.dma_start(ox_l[64:128, t, csl], tlo)
                if t < 3:
                    q_t, k_t = nq_t, nk_t

            # ---- out conv second half (+ sumA) ----
            for occ in range(8):
                for lh in range(2):
                    out_half_unit(occ, lh, 2, False)

    nc.compile()
    return nc


def _get_nc():
    if "nc" not in _CACHE:
        _CACHE["nc"] = _build_nc()
    return _CACHE["nc"]


def _f8split(arr):
    hi = arr.astype(F8)
    lo = (arr - hi.astype(np.float32)).astype(F8)
    return hi, lo


def _prep_inputs(query, key, value, key_padding_mask, attn_mask,
                 q_w, q_b, k_w, k_b, v_w, v_b, o_w, o_b):
    """Build the 8 per-core input maps (host-side shard + quantize)."""
    query = np.asarray(query, np.float32)
    key = np.asarray(key, np.float32)
    value = np.asarray(value, np.float32)
    kpm = np.asarray(key_padding_mask)
    attn_mask = np.asarray(attn_mask, np.float32)
    q_w = np.asarray(q_w, np.float32); q_b = np.asarray(q_b, np.float32)
    k_w = np.asarray(k_w, np.float32)
    v_w = np.asarray(v_w, np.float32)
    o_w = np.asarray(o_w, np.float32); o_b = np.asarray(o_b, np.float32)

    # attn_mask must be constant across query rows to fold into the key bias
    if not np.all(attn_mask == attn_mask[0:1, :]):
        raise NotImplementedError("attn_mask varying over query index unsupported")
    am_row = attn_mask[0]

    def conv_w_layout(w, occ, n_ci):
        # w: [C_out, C_in, K] -> [occ, p(ci), k, cc, m(c_out)]
        arr = w.reshape(occ, 128, n_ci, 128, KW).transpose(0, 3, 4, 2, 1)
        return _f8split(np.ascontiguousarray(arr))

    wq_h, wk_h, wv_h, wo_h, qb_h = [], [], [], [], []
    for hg in range(2):
        sl = slice(hg * HALF, (hg + 1) * HALF)
        wq_h.append(conv_w_layout(q_w[sl] * (SCALE * SWQ), 4, 8))
        wk_h.append(conv_w_layout(k_w[sl] * SWK, 4, 8))
        wv_h.append(conv_w_layout(v_w[sl] * SWV, 4, 8))
        wo_h.append(conv_w_layout(o_w[:, sl, :] * SWO, 8, 4))
        qb_h.append(np.ascontiguousarray(
            (q_b[sl] * (SCALE * SQ)).reshape(1, 512)).astype(BF16))

    xq_b, xk_b, xv_b, jb_b = [], [], [], []
    for b in range(B):
        xq_b.append(_f8split(
            np.ascontiguousarray(query[b].T * SX).reshape(8, 128, L)))
        xk_b.append(_f8split(
            np.ascontiguousarray(key[b].T * SX).reshape(8, 128, L)))
        xv_b.append(_f8split(
            np.ascontiguousarray(value[b].T * SX).reshape(8, 128, L)))
        jb = np.where(kpm[b], MASK_BIAS, 0.0).astype(np.float32) + am_row
        jb_b.append(np.ascontiguousarray(jb.reshape(8, 128).T).astype(np.float32))

    in_maps = []
    for c in range(NCORES):
        b, hg = c // 2, c % 2
        in_maps.append({
            "xqh": xq_b[b][0], "xql": xq_b[b][1],
            "xkh": xk_b[b][0], "xkl": xk_b[b][1],
            "xvh": xv_b[b][0], "xvl": xv_b[b][1],
            "wqh": wq_h[hg][0], "wql": wq_h[hg][1],
            "wkh": wk_h[hg][0], "wkl": wk_h[hg][1],
            "wvh": wv_h[hg][0], "wvl": wv_h[hg][1],
            "woh": wo_h[hg][0], "wol": wo_h[hg][1],
            "qb": qb_h[hg], "jb": jb_b[b],
        })
    return in_maps, (o_w, np.asarray(v_b, np.float32), o_b)


def _postprocess(parts, extras):
    """parts: list of 8 arrays [8,128,L] f32 -> full output [B, L, D] f32."""
    o_w, v_b, o_b = extras
    # v-bias contribution through the out conv (attention rows sum to 1):
    # interior columns see all 3 taps, edge columns lose one.
    a_full = o_w.sum(axis=2) @ v_b            # [D]
    a_l0 = a_full - o_w[:, :, 0] @ v_b        # l = 0 loses tap k=0
    a_lL = a_full - o_w[:, :, 2] @ v_b        # l = L-1 loses tap k=2
    out = np.empty((B, L, D), np.float32)
    for b in range(B):
        tot = (parts[2 * b] + parts[2 * b + 1]).reshape(D, L)
        tot = tot + o_b[:, None] + a_full[:, None]
        tot[:, 0] += a_l0 - a_full
        tot[:, -1] += a_lL - a_full
        out[b] = tot.T
    return out


def _run(in_maps, trace=False, **kw):
    from concourse import bass_utils
    nc = _get_nc()
    try:
        res = bass_utils.run_bass_kernel_spmd(
            nc, in_maps, core_ids=list(range(NCORES)), trace=trace, **kw)
    except ModuleNotFoundError:
        # NTFF profiling hook unavailable (axon client without axon.trn);
        # rerun without trace.
        res = bass_utils.run_bass_kernel_spmd(
            nc, in_maps, core_ids=list(range(NCORES)), trace=False, **kw)
    return res


def kernel(**inputs) -> np.ndarray:
    in_maps, extras = _prep_inputs(**inputs)
    res = _run(in_maps, trace=bool(int(os.environ.get("KERNEL_TRACE", "0"))))
    parts = [res.results[c]["out"] for c in range(NCORES)]
    out = _postprocess(parts, extras)
    if res.exec_time_ns is not None:
        print(f"HW exec time: {res.exec_time_ns} ns")
    return out


# revision 22
# speedup vs baseline: 1.3639x; 1.1898x over previous
"""ConvMultiheadAttention Trainium2 kernel (8 NeuronCores).

Sharding: core c = (batch b = c//2) x (head-group hg = c%2, 8 heads each).

All four convs run as fp8(e4m3) DoubleRow matmuls (256-wide contraction at
0.5 PE-cycles/row) with hi+lo residual splits of both weights and
activations for ~bf16 accuracy at 0.75x the bf16 PE cost:
  w*x ~= w_hi*x_hi + w_hi*x_lo + w_lo*x_hi   (lo terms at the same
  power-of-2 scale as hi, so all products accumulate in one psum group).
The V conv runs in transposed form (out[j, co]) so vT needs no PE
transposes. Scores are fp8 DoubleRow with q fully compensated (hi/lo as
the two k-tiles) and k raw-fp8 (hi duplicated): 2x cheaper than bf16,
~1% output error. AV + softmax stay bf16 (ones-row denominator trick).
The out conv is split into two cc-halves: the pairs-0/1 half runs inside
pair 3's exp window (PE filler), accumulating into SBUF; the pairs-2/3
half adds it back during the final psum->sbuf copy.

Host folds: attention scale + q-bias into q-conv weights/bias (bias enters
the conv psum via a ones-row bf16 matmul); k-bias dropped (softmax-
invariant); v-bias and o-bias applied on host after the gather.
"""

import os
import numpy as np
import ml_dtypes

BF16 = ml_dtypes.bfloat16
F8 = ml_dtypes.float8_e4m3

B, L, D = 4, 1024, 1024
NH, HD = 16, 64
KW = 3
NCORES = 8
HALF = D // 2  # channels per core half (8 heads)
SCALE = HD ** -0.5
MASK_BIAS = -30000.0

# power-of-2 scales: x inputs at 2^1; conv weights chosen so psum values
# land near sigma~37 (e4m3 max 240, so ~6 sigma of range)
SX = 2.0
SWQ = 2.0 ** 8   # on q_w * SCALE
SWK = 2.0 ** 5
SWV = 2.0 ** 5
SWO = 2.0 ** 5
SQ = SWQ * SX        # 2^9: scale of q values in fp8
SK = SWK * SX        # 2^6: scale of k values in fp8
SV = SWV * SX        # 2^6: scale of v / attention-output values
EXP_SCALE = 1.0 / (SQ * SK)   # 2^-15
OUT_SCALE = 1.0 / (SV * SWO)  # 2^-11: final psum -> true output

_CACHE = {}


def _build_nc():
    import concourse.bass as bass  # noqa: F401
    import concourse.tile as tile
    from concourse import bacc, mybir

    f32 = mybir.dt.float32
    bf16 = mybir.dt.bfloat16
    fp8 = mybir.dt.float8e4
    Act = mybir.ActivationFunctionType
    DR = mybir.MatmulPerfMode.DoubleRow
    Alu = mybir.AluOpType

    nc = bacc.Bacc(
        "TRN2",
        target_bir_lowering=False,
        debug=False,
        enable_asserts=False,
        num_devices=NCORES,
    )

    # ---- DRAM I/O ----
    def din(name, shape, dt=fp8):
        return nc.dram_tensor(name, shape, dt, kind="ExternalInput").ap()

    xq_d = [din("xqh", [128, 8, L]), din("xql", [128, 8, L])]
    xk_d = [din("xkh", [128, 8, L]), din("xkl", [128, 8, L])]
    xv_d = [din("xvh", [128, 8, L]), din("xvl", [128, 8, L])]
    wq_d = [din("wqh", [4, 128, KW, 8, 128]), din("wql", [4, 128, KW, 8, 128])]
    wk_d = [din("wkh", [4, 128, KW, 8, 128]), din("wkl", [4, 128, KW, 8, 128])]
    wv_d = [din("wvh", [4, 128, KW, 8, 128]), din("wvl", [4, 128, KW, 8, 128])]
    wo_d = [din("woh", [8, 128, KW, 4, 128]), din("wol", [8, 128, KW, 4, 128])]
    qb_d = din("qb", [1, 512], bf16)
    jb_d = din("jb", [128, 8], f32)
    out_d = nc.dram_tensor("out", [8, 128, L], f32, kind="ExternalOutput").ap()

    LP = L + 2  # padded length: col 0 and L+1 are zeros, data at 1..L

    with tile.TileContext(nc) as tc:
        with (
            tc.tile_pool(name="singles", bufs=1) as singles,
            tc.tile_pool(name="wpool", bufs=2) as wpool,
            tc.tile_pool(name="qk", bufs=2) as qkpool,
            tc.tile_pool(name="ppool", bufs=2) as ppool,
            tc.tile_pool(name="outp", bufs=4) as outp,
            tc.tile_pool(name="smalls", bufs=2) as smalls,
            tc.tile_pool(name="convp", bufs=2, space="PSUM") as convp,
            tc.tile_pool(name="scorep", bufs=2, space="PSUM") as scorep,
            tc.tile_pool(name="avp", bufs=2, space="PSUM") as avp,
        ):
            # ---- resident tiles ----
            qb_s = singles.tile([1, 512], bf16, tag="qb")
            nc.gpsimd.dma_start(qb_s, qb_d)
            jb_s = singles.tile([128, 8], f32, tag="jb")
            nc.gpsimd.dma_start(jb_s, jb_d)
            ones = singles.tile([1, 512], bf16, tag="ones")
            nc.vector.memset(ones, 1.0)

            def xload(tag, dpair, eng):
                """One big DMA per tensor into the padded tile, issued on
                `eng`'s queue so the input loads spread across HWDGEs."""
                hi = singles.tile([128, 8, LP], fp8, tag=tag + "h", name=tag + "h")
                lo = singles.tile([128, 8, LP], fp8, tag=tag + "l", name=tag + "l")
                for t in (hi, lo):
                    nc.vector.memset(t[:, :, 0:1], 0.0)
                    nc.vector.memset(t[:, :, L + 1 : L + 2], 0.0)
                for t, d in ((hi, dpair[0]), (lo, dpair[1])):
                    eng.dma_start(t[:, :, 1 : L + 1], d)
                return hi, lo

            xv_s = xload("xv", xv_d, nc.scalar)
            xq_s = xload("xq", xq_d, nc.gpsimd)
            xk_s = xload("xk", xk_d, nc.gpsimd)

            # vT[j_part, jc, head, 0:64] = v[h*64+d, j] * SV; col 64 = ones
            vT = singles.tile([128, 8, 8, 65], bf16, tag="vT")
            for h in range(8):
                nc.vector.memset(vT[:, :, h, 64:65], 1.0)
            from concourse.masks import make_identity
            ident = singles.tile([128, 128], bf16, tag="ident")
            make_identity(nc, ident)

            ox_h = singles.tile([128, 4, LP], fp8, tag="oxh")
            ox_l = singles.tile([128, 4, LP], fp8, tag="oxl")
            for t in (ox_h, ox_l):
                nc.vector.memset(t[:, :, 0:1], 0.0)
                nc.vector.memset(t[:, :, L + 1 : L + 2], 0.0)

            sumA = singles.tile([128, 16, 512], bf16, tag="sumA")

            def conv_dr(ps, wh, wl, xh, xl, lh, bias_lhsT=None):
                """512-col direct conv psum group: out cols [lh*512,+512),
                contraction 8 cc x 3 taps via fp8 DoubleRow hi/lo terms."""
                mms = []
                for k in (1, 0, 2):
                    base = lh * 512 + k
                    for c in range(0, 8, 2):
                        for wt, xt in ((wh, xh), (wh, xl), (wl, xh)):
                            mms.append((wt[:, k, c : c + 2, :],
                                        xt[:, c : c + 2, base : base + 512]))
                for i, (w_ap, x_ap) in enumerate(mms):
                    last = i == len(mms) - 1 and bias_lhsT is None
                    nc.tensor.matmul(ps, w_ap, x_ap, start=(i == 0),
                                     stop=last, perf_mode=DR)
                if bias_lhsT is not None:
                    nc.tensor.matmul(ps, bias_lhsT, ones, start=False,
                                     stop=True)

            # ---- V conv (direct, DR fp8) + PE transpose into vT ----
            # weight prefetch one occ ahead; transposes use the (idle) avp
            # psum ring; vT copies split across DVE and Act
            wv_tiles = {}

            def wv_fetch(occ):
                if occ > 3 or occ in wv_tiles:
                    return
                wvh = wpool.tile([128, KW, 8, 128], fp8, tag="wvh",
                                 name="wvht")
                wvl = wpool.tile([128, KW, 8, 128], fp8, tag="wvl",
                                 name="wvlt")
                nc.sync.dma_start(wvh, wv_d[0][occ])
                nc.sync.dma_start(wvl, wv_d[1][occ])
                wv_tiles[occ] = (wvh, wvl)

            wv_fetch(0)
            for occ in range(4):
                wv_fetch(occ + 1)
                wv_h, wv_l = wv_tiles.pop(occ)
                v_t = qkpool.tile([128, L], bf16, tag="v", name="vt")
                for lh in range(2):
                    ps = convp.tile([128, 512], f32, tag="cp")
                    conv_dr(ps, wv_h, wv_l, xv_s[0], xv_s[1], lh)
                    nc.scalar.activation(v_t[:, lh * 512 : (lh + 1) * 512],
                                         ps, Act.Copy)
                for lb in range(8):
                    tp = avp.tile([128, 128], bf16, tag="av")
                    nc.tensor.transpose(tp, v_t[:, lb * 128 : (lb + 1) * 128],
                                        ident)
                    nc.vector.tensor_copy(vT[:, lb, 2 * occ, 0:64],
                                          tp[:, 0:64])
                    nc.scalar.activation(vT[:, lb, 2 * occ + 1, 0:64],
                                         tp[:, 64:128], Act.Copy)

            # ---- per head-pair: pipelined q/k conv + attention ----
            def qk_conv_units(t):
                """(q_t, k_t, units): 4 closures each emitting one conv
                psum-group (~4us dense PE work) for pair t.
                q_t[:, 0, :] = q_hi, [:, 1, :] = q_lo (DoubleRow k-tiles);
                k_t[:, 0, :] = k_t[:, 1, :] = k_hi."""
                q_t = qkpool.tile([128, 2, L], fp8, tag="q", name=f"q{t}")
                # k: [ch, jc, ktile, m] so the scores stationary slice
                # k_t[d, jc, :, :] is ISA-contiguous (2, 128)
                k_t = qkpool.tile([128, 8, 2, 128], fp8, tag="k", name=f"k{t}")
                # weights DMA'd at call time (one pair ahead of use)
                state = {}
                for key, dp in (("wqh", wq_d[0]), ("wql", wq_d[1]),
                                ("wkh", wk_d[0]), ("wkl", wk_d[1])):
                    state[key] = wpool.tile([128, KW, 8, 128], fp8,
                                            tag=key, name=key + "t")
                    nc.sync.dma_start(state[key], dp[t])

                def unit(which, lh):
                    if which == "q":
                        ps = convp.tile([128, 512], f32, tag="cp")
                        sl = slice(lh * 512, (lh + 1) * 512)
                        conv_dr(ps, state["wqh"], state["wql"], xq_s[0],
                                xq_s[1], lh,
                                bias_lhsT=qb_s[0:1, t * 128 : (t + 1) * 128])
                        nc.vector.tensor_copy(q_t[:, 0, sl], ps)
                        nc.vector.tensor_sub(q_t[:, 1, sl], ps, q_t[:, 0, sl])
                    else:
                        ps = convp.tile([128, 4, 128], f32, tag="cp")
                        conv_dr(ps, state["wkh"], state["wkl"], xk_s[0],
                                xk_s[1], lh)
                        jsl = slice(lh * 4, (lh + 1) * 4)
                        nc.vector.tensor_copy(k_t[:, jsl, 0, :], ps)
                        nc.vector.tensor_copy(k_t[:, jsl, 1, :], ps)

                units = [lambda w=w, lh=lh: unit(w, lh)
                         for w in ("q", "k") for lh in range(2)]
                return q_t, k_t, units

            def qk_conv(t):
                q_t, k_t, units = qk_conv_units(t)
                for u in units:
                    u()
                return q_t, k_t

            # out-conv halves: cc pair (0,1) = head-pairs 0-1, (2,3) = 2-3
            wo_tiles = {}

            def wo_fetch(occ, half):
                """Prefetch wo weights for one occ (bufs=3 ring: current +
                two ahead)."""
                key = (occ, half)
                if occ > 7 or key in wo_tiles:
                    return
                woh = wpool.tile([128, KW, 4, 128], fp8, tag="woh",
                                 name="woht", bufs=3)
                wol = wpool.tile([128, KW, 4, 128], fp8, tag="wol",
                                 name="wolt", bufs=3)
                nc.sync.dma_start(woh, wo_d[0][occ])
                nc.sync.dma_start(wol, wo_d[1][occ])
                wo_tiles[key] = (woh, wol)

            def out_half_unit(occ, lh, cp0, first_half):
                """One psum group: out[occ, lh*512:+512] partial over o_x
                cc chunks [cp0, cp0+2)."""
                half = "h" if first_half else "H"
                if lh == 0:
                    wo_fetch(occ + 2, half)
                woh, wol = wo_tiles[(occ, half)]
                ps = convp.tile([128, 512], f32, tag="cp")
                mms = []
                for k in (1, 0, 2):
                    base = lh * 512 + k
                    for wt, xt in ((woh, ox_h), (woh, ox_l), (wol, ox_h)):
                        mms.append((wt[:, k, cp0 : cp0 + 2, :],
                                    xt[:, cp0 : cp0 + 2, base : base + 512]))
                for i, (w_ap, x_ap) in enumerate(mms):
                    nc.tensor.matmul(ps, w_ap, x_ap, start=(i == 0),
                                     stop=(i == len(mms) - 1), perf_mode=DR)
                idx = occ * 2 + lh
                if first_half:
                    nc.vector.tensor_scalar_mul(sumA[:, idx, :], ps,
                                                OUT_SCALE)
                else:
                    o_t = outp.tile([128, 512], f32, tag="osb")
                    nc.vector.scalar_tensor_tensor(
                        o_t, ps, OUT_SCALE, sumA[:, idx, :],
                        Alu.mult, Alu.add)
                    nc.sync.dma_start(
                        out_d[occ, :, lh * 512 : (lh + 1) * 512], o_t)

            q_t, k_t = qk_conv(0)
            for t in range(4):
                if t < 3:
                    nq_t, nk_t, conv_units = qk_conv_units(t + 1)
                else:
                    wo_fetch(0, "h")
                    wo_fetch(1, "h")
                    conv_units = [
                        (lambda occ=occ, lh=lh:
                         out_half_unit(occ, lh, 0, True))
                        for occ in range(8) for lh in range(2)]
                p_pair = []
                for jc in range(8):
                    sps_pair = [scorep.tile([128, L], f32, tag="score",
                                            name=f"sps{hh2}")
                                for hh2 in range(2)]
                    if jc == 0:
                        p_pair = [ppool.tile([128, 8, L], bf16, tag="p",
                                             name=f"p{hh2}")
                                  for hh2 in range(2)]
                    for ih in range(2):
                        for hh in range(2):
                            base = hh * 64
                            nc.tensor.matmul(
                                sps_pair[hh][:, ih * 512 : (ih + 1) * 512],
                                k_t[base : base + 64, jc, :, :],
                                q_t[base : base + 64, :,
                                    ih * 512 : (ih + 1) * 512],
                                start=True, stop=True, perf_mode=DR,
                            )
                    for hh in range(2):
                        nc.scalar.activation(
                            p_pair[hh][:, jc, :], sps_pair[hh], Act.Exp,
                            bias=jb_s[:, jc : jc + 1], scale=EXP_SCALE,
                        )
                    if t < 3:
                        # one conv psum-group after every other jc-group
                        if jc % 2 == 1 and conv_units:
                            conv_units.pop(0)()
                    else:
                        # two out-conv first-half groups per jc
                        for _ in range(2):
                            if conv_units:
                                conv_units.pop(0)()
                # AV + normalize + fp8 hi/lo split of o_x
                for hh in range(2):
                    h = 2 * t + hh
                    base = hh * 64
                    for ih in range(2):
                        avps = avp.tile([65, 512], f32, tag="av")
                        for jc in range(8):
                            nc.tensor.matmul(
                                avps,
                                vT[:, jc, h, :],
                                p_pair[hh][:, jc,
                                           ih * 512 : (ih + 1) * 512],
                                start=(jc == 0), stop=(jc == 7),
                            )
                        r_t = smalls.tile([1, 512], f32, tag="r")
                        nc.vector.reciprocal(r_t, avps[64:65, :])
                        bc_t = smalls.tile([64, 512], f32, tag="bc")
                        nc.gpsimd.partition_broadcast(bc_t, r_t)
                        tmp = smalls.tile([64, 512], bf16, tag="tmp")
                        nc.vector.tensor_mul(tmp, avps[0:64, :], bc_t)
                        csl = slice(1 + ih * 512, 1 + (ih + 1) * 512)
                        if hh == 0:
                            nc.vector.tensor_copy(ox_h[0:64, t, csl], tmp)
                            nc.vector.tensor_sub(ox_l[0:64, t, csl], tmp,
                                                 ox_h[0:64, t, csl])
                        else:
                            thi = smalls.tile([64, 512], fp8, tag="thi")
                            tlo = smalls.tile([64, 512], fp8, tag="tlo")
                            nc.vector.tensor_copy(thi, tmp)
                            nc.vector.tensor_sub(tlo, tmp, thi)
                            nc.sync.dma_start(ox_h[64:128, t, csl], thi)
                            nc.sync.dma_start(ox_l[64:128, t, csl], tlo)
                if t < 3:
                    q_t, k_t = nq_t, nk_t

            # ---- out conv second half (+ sumA) ----
            wo_fetch(0, "H")
            wo_fetch(1, "H")
            for occ in range(8):
                for lh in range(2):
                    out_half_unit(occ, lh, 2, False)

    nc.compile()
    return nc


def _get_nc():
    if "nc" not in _CACHE:
        _CACHE["nc"] = _build_nc()
    return _CACHE["nc"]


def _f8split(arr):
    hi = arr.astype(F8)
    lo = (arr - hi.astype(np.float32)).astype(F8)
    return hi, lo


def _prep_inputs(query, key, value, key_padding_mask, attn_mask,
                 q_w, q_b, k_w, k_b, v_w, v_b, o_w, o_b):
    """Build the 8 per-core input maps (host-side shard + quantize)."""
    query = np.asarray(query, np.float32)
    key = np.asarray(key, np.float32)
    value = np.asarray(value, np.float32)
    kpm = np.asarray(key_padding_mask)
    attn_mask = np.asarray(attn_mask, np.float32)
    q_w = np.asarray(q_w, np.float32); q_b = np.asarray(q_b, np.float32)
    k_w = np.asarray(k_w, np.float32)
    v_w = np.asarray(v_w, np.float32)
    o_w = np.asarray(o_w, np.float32); o_b = np.asarray(o_b, np.float32)

    # attn_mask must be constant across query rows to fold into the key bias
    if not np.all(attn_mask == attn_mask[0:1, :]):
        raise NotImplementedError("attn_mask varying over query index unsupported")
    am_row = attn_mask[0]

    def conv_w_layout(w, occ, n_ci):
        # w: [C_out, C_in, K] -> [occ, p(ci), k, cc, m(c_out)]
        arr = w.reshape(occ, 128, n_ci, 128, KW).transpose(0, 3, 4, 2, 1)
        return _f8split(np.ascontiguousarray(arr))

    wq_h, wk_h, wv_h, wo_h, qb_h = [], [], [], [], []
    for hg in range(2):
        sl = slice(hg * HALF, (hg + 1) * HALF)
        wq_h.append(conv_w_layout(q_w[sl] * (SCALE * SWQ), 4, 8))
        wk_h.append(conv_w_layout(k_w[sl] * SWK, 4, 8))
        wv_h.append(conv_w_layout(v_w[sl] * SWV, 4, 8))
        wo_h.append(conv_w_layout(o_w[:, sl, :] * SWO, 8, 4))
        qb_h.append(np.ascontiguousarray(
            (q_b[sl] * (SCALE * SQ)).reshape(1, 512)).astype(BF16))

    def xprep(x):
        # [L, D] -> [128 p(ci%128), 8 cc(ci//128), L]
        arr = (x.T * SX).reshape(8, 128, L).transpose(1, 0, 2)
        return _f8split(np.ascontiguousarray(arr))

    xq_b, xk_b, xv_b, jb_b = [], [], [], []
    for b in range(B):
        xq_b.append(xprep(query[b]))
        xk_b.append(xprep(key[b]))
        xv_b.append(xprep(value[b]))
        jb = np.where(kpm[b], MASK_BIAS, 0.0).astype(np.float32) + am_row
        jb_b.append(np.ascontiguousarray(jb.reshape(8, 128).T).astype(np.float32))

    in_maps = []
    for c in range(NCORES):
        b, hg = c // 2, c % 2
        in_maps.append({
            "xqh": xq_b[b][0], "xql": xq_b[b][1],
            "xkh": xk_b[b][0], "xkl": xk_b[b][1],
            "xvh": xv_b[b][0], "xvl": xv_b[b][1],
            "wqh": wq_h[hg][0], "wql": wq_h[hg][1],
            "wkh": wk_h[hg][0], "wkl": wk_h[hg][1],
            "wvh": wv_h[hg][0], "wvl": wv_h[hg][1],
            "woh": wo_h[hg][0], "wol": wo_h[hg][1],
            "qb": qb_h[hg], "jb": jb_b[b],
        })
    return in_maps, (o_w, np.asarray(v_b, np.float32), o_b)


def _postprocess(parts, extras):
    """parts: list of 8 arrays [8,128,L] f32 -> full output [B, L, D] f32."""
    o_w, v_b, o_b = extras
    # v-bias contribution through the out conv (attention rows sum to 1):
    # interior columns see all 3 taps, edge columns lose one.
    a_full = o_w.sum(axis=2) @ v_b            # [D]
    a_l0 = a_full - o_w[:, :, 0] @ v_b        # l = 0 loses tap k=0
    a_lL = a_full - o_w[:, :, 2] @ v_b        # l = L-1 loses tap k=2
    out = np.empty((B, L, D), np.float32)
    for b in range(B):
        tot = (parts[2 * b] + parts[2 * b + 1]).reshape(D, L)
        tot = tot + o_b[:, None] + a_full[:, None]
        tot[:, 0] += a_l0 - a_full
        tot[:, -1] += a_lL - a_full
        out[b] = tot.T
    return out


def _run(in_maps, trace=False, **kw):
    from concourse import bass_utils
    nc = _get_nc()
    try:
        res = bass_utils.run_bass_kernel_spmd(
            nc, in_maps, core_ids=list(range(NCORES)), trace=trace, **kw)
    except ModuleNotFoundError:
        # NTFF profiling hook unavailable (axon client without axon.trn);
        # rerun without trace.
        res = bass_utils.run_bass_kernel_spmd(
            nc, in_maps, core_ids=list(range(NCORES)), trace=False, **kw)
    return res


def kernel(**inputs) -> np.ndarray:
    in_maps, extras = _prep_inputs(**inputs)
    res = _run(in_maps, trace=bool(int(os.environ.get("KERNEL_TRACE", "0"))))
    parts = [res.results[c]["out"] for c in range(NCORES)]
    out = _postprocess(parts, extras)
    if res.exec_time_ns is not None:
        print(f"HW exec time: {res.exec_time_ns} ns")
    return out
